# revision 12
# baseline (speedup 1.0000x reference)
"""Distributed KNN-retrieval kernel for 8 Trainium2 NeuronCores.

Reference computation:
  protos = MLP(input_state)                 # [5, 20]
  dists[s, n] = || candidate_docs[n] - protos[s] ||_2
  top-100 smallest per slate -> (candidates [500, 20], indices [500])

Device strategy (SPMD over 8 cores, candidate_docs row-sharded):
  - Host pre-packs each 125k-row shard as fp16 [120, 20992]: column r holds 6
    consecutive docs (6 docs x 20 dims = 120 partitions).
  - On-device MLP (fp32) reproduces the reference policy net and emits the
    proto matrix transposed [20, 5].
  - A block-diagonal stationary [120, 36] (6 copies of protoT on the diagonal
    + 6 ones-columns) lets one matmul over a [120, 512] moving slice produce
    dot products for 6 docs x 5 slates, and a second matmul over the squared
    docs produce the 6 doc norms.
  - Scores s = ||c||^2 - 2 c.p are assembled on host (fp16 is plenty for
    candidate SELECTION: validated max local rank of a true top-100 member
    is ~20), then the final top-100 is recomputed exactly (same jnp ops as
    the reference) over a pooled candidate set, so the returned indices and
    rows match the reference bitwise.
"""

import sys

sys.path.insert(0, "/opt/trn_rl_repo")

import numpy as np

# ---- problem constants (hardcoded per harness contract) ----
D = 20
SLATE = 5
TOPK = 100
N_DOCS = 1_000_000
HID = 256
SLOPE = 0.01

NCORES = 8
SHARD = N_DOCS // NCORES          # 125000
PACK = 6                          # docs packed per column
PDIM = PACK * D                   # 120 partitions
NCOL = 20992                      # columns per core: 6*20992 = 125952 >= 125000
PAD_SHARD = PACK * NCOL           # 125952
CHUNK = 4096                      # moving columns per DMA chunk / psum tile
NFULL = 5                         # full chunks; tail = 512
TAIL = NCOL - NFULL * CHUNK       # 512
NCHUNK = NFULL + 1
POOL_K = 2048                     # per-slate host pool size for exact rerank

_CACHE = {}


def _build_nc():
    """Build the per-core Bass/Tile graph (identical on all 8 cores)."""
    from concourse import bacc, mybir

    f32 = mybir.dt.float32
    f16 = mybir.dt.float16
    Act = mybir.ActivationFunctionType
    Alu = mybir.AluOpType

    import concourse.tile as tile

    nc = bacc.Bacc("TRN2", target_bir_lowering=False)

    docsT = nc.declare_dram_parameter("docsT", [PDIM, NCOL], f16, isOutput=False)
    x_in = nc.declare_dram_parameter("input_state", [D, 1], f32, isOutput=False)
    w1t = nc.declare_dram_parameter("W1T", [D, HID], f32, isOutput=False)
    b1_in = nc.declare_dram_parameter("b1r", [128, 2], f32, isOutput=False)
    w2t = nc.declare_dram_parameter("W2T", [HID, HID], f32, isOutput=False)
    b2_in = nc.declare_dram_parameter("b2r", [128, 2], f32, isOutput=False)
    w3t = nc.declare_dram_parameter("W3T", [HID, SLATE * D], f32, isOutput=False)
    b3t_in = nc.declare_dram_parameter("b3t", [D, SLATE], f32, isOutput=False)
    ones_in = nc.declare_dram_parameter("blockones", [PDIM, PACK], f16, isOutput=False)
    out_d = nc.declare_dram_parameter("out", [NCHUNK, 102, CHUNK // 2], f16, isOutput=True)

    with tile.TileContext(nc) as tc:
        with (
            tc.tile_pool(name="const", bufs=1) as const,
            tc.tile_pool(name="mlp", bufs=1) as mlp,
            tc.tile_pool(name="docs", bufs=3) as dpool,
            tc.tile_pool(name="sq", bufs=3) as qpool,
            tc.tile_pool(name="stg", bufs=3) as gpool,
            tc.tile_pool(name="ps", bufs=2, space="PSUM") as pspool,
        ):
            # ---------------- setup: weights + MLP ----------------
            w1 = const.tile([D, HID], f32)
            nc.sync.dma_start(out=w1[:], in_=w1t[:])
            w2 = const.tile([128, 2, HID], f32)
            nc.sync.dma_start(out=w2[:, 0, :], in_=w2t[0:128, :])
            nc.sync.dma_start(out=w2[:, 1, :], in_=w2t[128:256, :])
            w3 = const.tile([128, 2, SLATE * D], f32)
            nc.sync.dma_start(out=w3[:, 0, :], in_=w3t[0:128, :])
            nc.sync.dma_start(out=w3[:, 1, :], in_=w3t[128:256, :])
            b1 = const.tile([128, 2], f32)
            nc.sync.dma_start(out=b1[:], in_=b1_in[:])
            b2 = const.tile([128, 2], f32)
            nc.sync.dma_start(out=b2[:], in_=b2_in[:])
            b3 = const.tile([D, SLATE], f32)
            nc.sync.dma_start(out=b3[:], in_=b3t_in[:])
            xs = const.tile([D, 1], f32)
            nc.sync.dma_start(out=xs[:], in_=x_in[:])

            # stationary [120, 64]: cols 0-29 proto block-diagonal (30-31 zero),
            # cols 32-37 block-ones (38-63 zero).  Both matmul stationaries are
            # 32 wide so every PSUM partition gets written (no uninit reads).
            stat = const.tile([PDIM, 64], f16)
            nc.vector.memset(stat[:], 0.0)
            nc.sync.dma_start(out=stat[:, 32 : 32 + PACK], in_=ones_in[:])

            def leaky(dst, src):
                t = mlp.tile(list(src.shape), f32, tag="lk")
                nc.vector.tensor_scalar_mul(t[:], src, SLOPE)
                nc.vector.tensor_tensor(out=dst, in0=src, in1=t[:], op=Alu.max)

            # layer 1: h1 = leaky(x @ W1.T + b1) as [128, 2]
            ps1 = pspool.tile([128, 2], f32, tag="ps")
            for m in range(2):
                nc.tensor.matmul(
                    out=ps1[:, m : m + 1],
                    lhsT=w1[:, 128 * m : 128 * (m + 1)],
                    rhs=xs[:],
                    start=True,
                    stop=True,
                )
            z1 = mlp.tile([128, 2], f32, tag="z1")
            nc.vector.tensor_tensor(out=z1[:], in0=ps1[:], in1=b1[:], op=Alu.add)
            h1 = mlp.tile([128, 2], f32, tag="h1")
            leaky(h1[:], z1[:])

            # layer 2
            ps2 = pspool.tile([128, 2], f32, tag="ps")
            for m in range(2):
                for k in range(2):
                    nc.tensor.matmul(
                        out=ps2[:, m : m + 1],
                        lhsT=w2[:, k, 128 * m : 128 * (m + 1)],
                        rhs=h1[:, k : k + 1],
                        start=(k == 0),
                        stop=(k == 1),
                    )
            z2 = mlp.tile([128, 2], f32, tag="z2")
            nc.vector.tensor_tensor(out=z2[:], in0=ps2[:], in1=b2[:], op=Alu.add)
            h2 = mlp.tile([128, 2], f32, tag="h2")
            leaky(h2[:], z2[:])

            # layer 3: emit protoT [20, 5] directly (column j = proto_j)
            ps3 = pspool.tile([D, SLATE], f32, tag="ps")
            for j in range(SLATE):
                for k in range(2):
                    nc.tensor.matmul(
                        out=ps3[:, j : j + 1],
                        lhsT=w3[:, k, D * j : D * (j + 1)],
                        rhs=h2[:, k : k + 1],
                        start=(k == 0),
                        stop=(k == 1),
                    )
            z3 = mlp.tile([D, SLATE], f32, tag="z3")
            nc.vector.tensor_tensor(out=z3[:], in0=ps3[:], in1=b3[:], op=Alu.add)
            pt32 = mlp.tile([D, SLATE], f32, tag="pt32")
            leaky(pt32[:], z3[:])
            pt16 = mlp.tile([D, SLATE], f16, tag="pt16")
            nc.vector.tensor_copy(out=pt16[:], in_=pt32[:])

            # place 6 copies of protoT on the block diagonal of `stat`
            for g in range(PACK):
                nc.sync.dma_start(
                    out=stat[D * g : D * (g + 1), SLATE * g : SLATE * (g + 1)],
                    in_=pt16[:],
                )

            # ---------------- main loop over doc chunks ----------------
            for c in range(NCHUNK):
                cols = CHUNK if c < NFULL else TAIL
                nslice = cols // 512
                nbank = (nslice + 1) // 2
                prows = 128 if nslice > 1 else 64
                dck = dpool.tile([PDIM, cols], f16, tag="d")
                # split each chunk across both HWDGE rings (SP + ACT issue)
                half = cols // 2
                nc.sync.dma_start(
                    out=dck[:, 0:half],
                    in_=docsT[:, CHUNK * c : CHUNK * c + half],
                )
                nc.scalar.dma_start(
                    out=dck[:, half:cols],
                    in_=docsT[:, CHUNK * c + half : CHUNK * c + cols],
                )
                sck = qpool.tile([PDIM, cols], f16, tag="q")
                nc.vector.tensor_tensor(
                    out=sck[:], in0=dck[:], in1=dck[:], op=Alu.mult
                )

                ps = pspool.tile([prows, 512 * nbank], f32, tag="ps")
                for s in range(nslice):
                    pe = s % 2
                    bank = s // 2
                    mv = slice(512 * s, 512 * (s + 1))
                    pc = slice(512 * bank, 512 * (bank + 1))
                    nc.tensor.matmul(
                        out=ps[64 * pe : 64 * pe + 32, pc],
                        lhsT=stat[:, 0:32],
                        rhs=dck[:, mv],
                        start=True,
                        stop=True,
                        tile_position=(0, 64 * pe),
                    )
                    nc.tensor.matmul(
                        out=ps[64 * pe + 32 : 64 * pe + 64, pc],
                        lhsT=stat[:, 32:64],
                        rhs=sck[:, mv],
                        start=True,
                        stop=True,
                        tile_position=(0, 64 * pe + 32),
                    )
                orow = min(prows, 102)
                stg = gpool.tile([orow, 512 * nbank], f16, tag="s")
                nc.scalar.activation(stg[:], ps[0:orow, :], Act.Copy)
                nc.gpsimd.dma_start(
                    out=out_d[c, 0:orow, 0 : 512 * nbank], in_=stg[:]
                )

    return nc


def _get_nc():
    if "nc" not in _CACHE:
        nc = _build_nc()
        nc.finalize()  # Bacc: reg-alloc + codegen passes + freeze
        _CACHE["nc"] = nc
    return _CACHE["nc"]


def make_in_maps(input_state, candidate_docs, W1, b1, W2, b2, W3, b3):
    """Shard + lay out host inputs for the 8 cores."""
    blockones = np.zeros((PDIM, PACK), np.float16)
    for g in range(PACK):
        blockones[D * g : D * (g + 1), g] = 1.0
    common = {
        "input_state": np.ascontiguousarray(
            input_state.reshape(D, 1).astype(np.float32)
        ),
        "W1T": np.ascontiguousarray(W1.T.astype(np.float32)),
        "b1r": np.ascontiguousarray(b1.astype(np.float32).reshape(2, 128).T),
        "W2T": np.ascontiguousarray(W2.T.astype(np.float32)),
        "b2r": np.ascontiguousarray(b2.astype(np.float32).reshape(2, 128).T),
        "W3T": np.ascontiguousarray(W3.T.astype(np.float32)),
        "b3t": np.ascontiguousarray(b3.astype(np.float32).reshape(SLATE, D).T),
        "blockones": blockones,
    }
    in_maps = []
    for i in range(NCORES):
        shard = candidate_docs[i * SHARD : (i + 1) * SHARD]
        pad = np.zeros((PAD_SHARD, D), np.float16)
        pad[:SHARD] = shard.astype(np.float16)
        docsT = np.ascontiguousarray(pad.reshape(NCOL, PDIM).T)
        in_maps.append({"docsT": docsT, **common})
    return in_maps


def decode_scores(out_arr):
    """[NCHUNK, 128, CHUNK//2] fp16 device output -> per-core scores [SHARD, 5]."""
    dots = np.empty((NCOL, PACK, SLATE), np.float32)
    norms = np.empty((NCOL, PACK), np.float32)
    for c in range(NCHUNK):
        cols = CHUNK if c < NFULL else TAIL
        nslice = cols // 512
        nbank = (nslice + 1) // 2
        blk = (
            out_arr[c][:, : 512 * nbank]
            .astype(np.float32)
            .reshape(102, nbank, 512)
        )
        for s in range(nslice):
            pe = s % 2
            bank = s // 2
            n0 = CHUNK * c + 512 * s
            d = blk[64 * pe : 64 * pe + 30, bank].reshape(PACK, SLATE, 512)
            nr = blk[64 * pe + 32 : 64 * pe + 38, bank]
            dots[n0 : n0 + 512] = d.transpose(2, 0, 1)
            norms[n0 : n0 + 512] = nr.T
    scores = norms[:, :, None] - 2.0 * dots
    return scores.reshape(PAD_SHARD, SLATE)[:SHARD]


def run_device(inputs):
    """Run the 8-core kernel; returns (scores [N_DOCS, 5] fp32, results)."""
    from concourse.bass_utils import run_bass_kernel_spmd

    nc = _get_nc()
    in_maps = make_in_maps(**inputs)
    res = run_bass_kernel_spmd(nc, in_maps, core_ids=list(range(NCORES)))
    parts = [decode_scores(np.asarray(res.results[i]["out"])) for i in range(NCORES)]
    return np.concatenate(parts, axis=0), res


def finalize(scores, input_state, candidate_docs, W1, b1, W2, b2, W3, b3):
    """Host pool + exact rerank replicating the reference ops bitwise."""
    import jax
    import jax.numpy as jnp

    pool = []
    for j in range(SLATE):
        pool.append(np.argpartition(scores[:, j], POOL_K)[:POOL_K])
    pool = np.unique(np.concatenate(pool))  # sorted ascending
    cpu = jax.devices("cpu")[0]
    with jax.default_device(cpu):
        x = jnp.asarray(input_state)
        for W, b in ((W1, b1), (W2, b2), (W3, b3)):
            x = jax.nn.leaky_relu(x @ jnp.asarray(W).T + jnp.asarray(b),
                                  negative_slope=SLOPE)
        proto_slate = x.reshape(SLATE, D)
        docs_pool = jnp.asarray(candidate_docs[pool])
        diff = docs_pool[None, :, :] - proto_slate[:, None, :]
        dists = jnp.sqrt(jnp.sum(diff * diff, axis=-1))
        _, idx = jax.lax.top_k(-dists, TOPK)
        idx = np.asarray(idx)
    indices = pool[idx].reshape(-1).astype(np.int32)
    candidates = candidate_docs[indices]
    return candidates, indices


def kernel(**inputs):
    inputs = {k: np.asarray(v) for k, v in inputs.items()}
    scores, _ = run_device(inputs)
    return finalize(scores, **inputs)


# revision 13
# speedup vs baseline: 1.0989x; 1.0989x over previous
"""Distributed KNN-retrieval kernel for 8 Trainium2 NeuronCores.

Reference computation:
  protos = MLP(input_state)                 # [5, 20]
  dists[s, n] = || candidate_docs[n] - protos[s] ||_2
  top-100 smallest per slate -> (candidates [500, 20], indices [500])

Device strategy (SPMD over 8 cores, candidate_docs row-sharded):
  - Host pre-packs each 125k-row shard as fp16 [120, 20992]: column r holds 6
    consecutive docs (6 docs x 20 dims = 120 partitions).
  - On-device MLP (fp32) reproduces the reference policy net and emits the
    proto matrix transposed [20, 5].
  - A block-diagonal stationary [120, 36] (6 copies of protoT on the diagonal
    + 6 ones-columns) lets one matmul over a [120, 512] moving slice produce
    dot products for 6 docs x 5 slates, and a second matmul over the squared
    docs produce the 6 doc norms.
  - Scores s = ||c||^2 - 2 c.p are assembled on host (fp16 is plenty for
    candidate SELECTION: validated max local rank of a true top-100 member
    is ~20), then the final top-100 is recomputed exactly (same jnp ops as
    the reference) over a pooled candidate set, so the returned indices and
    rows match the reference bitwise.
"""

import sys

sys.path.insert(0, "/opt/trn_rl_repo")

import numpy as np

# ---- problem constants (hardcoded per harness contract) ----
D = 20
SLATE = 5
TOPK = 100
N_DOCS = 1_000_000
HID = 256
SLOPE = 0.01

NCORES = 8
SHARD = N_DOCS // NCORES          # 125000
PACK = 6                          # docs packed per column
PDIM = PACK * D                   # 120 partitions
NCOL = 20992                      # columns per core: 6*20992 = 125952 >= 125000
PAD_SHARD = PACK * NCOL           # 125952
CHUNK = 4096                      # moving columns per DMA chunk / psum tile
NFULL = 5                         # full chunks; tail = 512
TAIL = NCOL - NFULL * CHUNK       # 512
NCHUNK = NFULL + 1
POOL_K = 2048                     # per-slate host pool size for exact rerank

_CACHE = {}


def _build_nc():
    """Build the per-core Bass/Tile graph (identical on all 8 cores)."""
    from concourse import bacc, mybir

    f32 = mybir.dt.float32
    f16 = mybir.dt.float16
    Act = mybir.ActivationFunctionType
    Alu = mybir.AluOpType

    import concourse.tile as tile

    nc = bacc.Bacc("TRN2", target_bir_lowering=False)

    docsT = nc.declare_dram_parameter("docsT", [128, NCOL], f16, isOutput=False)
    x_in = nc.declare_dram_parameter("input_state", [D, 1], f32, isOutput=False)
    w1t = nc.declare_dram_parameter("W1T", [D, HID], f32, isOutput=False)
    b1_in = nc.declare_dram_parameter("b1r", [128, 2], f32, isOutput=False)
    w2t = nc.declare_dram_parameter("W2T", [HID, HID], f32, isOutput=False)
    b2_in = nc.declare_dram_parameter("b2r", [128, 2], f32, isOutput=False)
    w3t = nc.declare_dram_parameter("W3T", [HID, SLATE * D], f32, isOutput=False)
    b3t_in = nc.declare_dram_parameter("b3t", [D, SLATE], f32, isOutput=False)
    ones_in = nc.declare_dram_parameter("blockones", [PDIM, PACK], f16, isOutput=False)
    out_d = nc.declare_dram_parameter("out", [NCHUNK, 128, CHUNK // 2], f16, isOutput=True)

    with tile.TileContext(nc) as tc:
        with (
            tc.tile_pool(name="const", bufs=1) as const,
            tc.tile_pool(name="mlp", bufs=1) as mlp,
            tc.tile_pool(name="docs", bufs=3) as dpool,
            tc.tile_pool(name="sq", bufs=3) as qpool,
            tc.tile_pool(name="stg", bufs=3) as gpool,
            tc.tile_pool(name="ps", bufs=2, space="PSUM") as pspool,
        ):
            # ---------------- setup: weights + MLP ----------------
            w1 = const.tile([D, HID], f32)
            nc.gpsimd.dma_start(out=w1[:], in_=w1t[:])
            w2 = const.tile([128, 2, HID], f32)
            nc.gpsimd.dma_start(out=w2[:, 0, :], in_=w2t[0:128, :])
            nc.gpsimd.dma_start(out=w2[:, 1, :], in_=w2t[128:256, :])
            w3 = const.tile([128, 2, SLATE * D], f32)
            nc.gpsimd.dma_start(out=w3[:, 0, :], in_=w3t[0:128, :])
            nc.gpsimd.dma_start(out=w3[:, 1, :], in_=w3t[128:256, :])
            b1 = const.tile([128, 2], f32)
            nc.gpsimd.dma_start(out=b1[:], in_=b1_in[:])
            b2 = const.tile([128, 2], f32)
            nc.gpsimd.dma_start(out=b2[:], in_=b2_in[:])
            b3 = const.tile([D, SLATE], f32)
            nc.gpsimd.dma_start(out=b3[:], in_=b3t_in[:])
            xs = const.tile([D, 1], f32)
            nc.gpsimd.dma_start(out=xs[:], in_=x_in[:])

            # stationary [120, 64]: cols 0-29 proto block-diagonal (30-31 zero),
            # cols 32-37 block-ones (38-63 zero).  Both matmul stationaries are
            # 32 wide so every PSUM partition gets written (no uninit reads).
            stat = const.tile([PDIM, 64], f16)
            nc.vector.memset(stat[:], 0.0)
            nc.gpsimd.dma_start(out=stat[:, 32 : 32 + PACK], in_=ones_in[:])

            def leaky(dst, src):
                t = mlp.tile(list(src.shape), f32, tag="lk")
                nc.vector.tensor_scalar_mul(t[:], src, SLOPE)
                nc.vector.tensor_tensor(out=dst, in0=src, in1=t[:], op=Alu.max)

            # layer 1: h1 = leaky(x @ W1.T + b1) as [128, 2]
            ps1 = pspool.tile([128, 2], f32, tag="ps")
            for m in range(2):
                nc.tensor.matmul(
                    out=ps1[:, m : m + 1],
                    lhsT=w1[:, 128 * m : 128 * (m + 1)],
                    rhs=xs[:],
                    start=True,
                    stop=True,
                )
            z1 = mlp.tile([128, 2], f32, tag="z1")
            nc.vector.tensor_tensor(out=z1[:], in0=ps1[:], in1=b1[:], op=Alu.add)
            h1 = mlp.tile([128, 2], f32, tag="h1")
            leaky(h1[:], z1[:])

            # layer 2
            ps2 = pspool.tile([128, 2], f32, tag="ps")
            for m in range(2):
                for k in range(2):
                    nc.tensor.matmul(
                        out=ps2[:, m : m + 1],
                        lhsT=w2[:, k, 128 * m : 128 * (m + 1)],
                        rhs=h1[:, k : k + 1],
                        start=(k == 0),
                        stop=(k == 1),
                    )
            z2 = mlp.tile([128, 2], f32, tag="z2")
            nc.vector.tensor_tensor(out=z2[:], in0=ps2[:], in1=b2[:], op=Alu.add)
            h2 = mlp.tile([128, 2], f32, tag="h2")
            leaky(h2[:], z2[:])

            # layer 3: emit protoT [20, 5] directly (column j = proto_j)
            ps3 = pspool.tile([D, SLATE], f32, tag="ps")
            for j in range(SLATE):
                for k in range(2):
                    nc.tensor.matmul(
                        out=ps3[:, j : j + 1],
                        lhsT=w3[:, k, D * j : D * (j + 1)],
                        rhs=h2[:, k : k + 1],
                        start=(k == 0),
                        stop=(k == 1),
                    )
            z3 = mlp.tile([D, SLATE], f32, tag="z3")
            nc.vector.tensor_tensor(out=z3[:], in0=ps3[:], in1=b3[:], op=Alu.add)
            pt32 = mlp.tile([D, SLATE], f32, tag="pt32")
            leaky(pt32[:], z3[:])
            pt16 = mlp.tile([D, SLATE], f16, tag="pt16")
            nc.vector.tensor_copy(out=pt16[:], in_=pt32[:])

            # place 6 copies of protoT on the block diagonal of `stat`
            for g in range(PACK):
                nc.gpsimd.dma_start(
                    out=stat[D * g : D * (g + 1), SLATE * g : SLATE * (g + 1)],
                    in_=pt16[:],
                )

            # ---------------- main loop over doc chunks ----------------
            for c in range(NCHUNK):
                cols = CHUNK if c < NFULL else TAIL
                nslice = cols // 512
                nbank = (nslice + 1) // 2
                prows = 128 if nslice > 1 else 64
                dck = dpool.tile([128, cols], f16, tag="d")
                nc.sync.dma_start(
                    out=dck[:], in_=docsT[:, CHUNK * c : CHUNK * c + cols]
                )
                sck = qpool.tile([PDIM, cols], f16, tag="q")
                nc.vector.tensor_tensor(
                    out=sck[:], in0=dck[0:PDIM, :], in1=dck[0:PDIM, :], op=Alu.mult
                )

                ps = pspool.tile([prows, 512 * nbank], f32, tag="ps")
                for s in range(nslice):
                    pe = s % 2
                    bank = s // 2
                    mv = slice(512 * s, 512 * (s + 1))
                    pc = slice(512 * bank, 512 * (bank + 1))
                    nc.tensor.matmul(
                        out=ps[64 * pe : 64 * pe + 32, pc],
                        lhsT=stat[:, 0:32],
                        rhs=dck[0:PDIM, mv],
                        start=True,
                        stop=True,
                        tile_position=(0, 64 * pe),
                    )
                    nc.tensor.matmul(
                        out=ps[64 * pe + 32 : 64 * pe + 64, pc],
                        lhsT=stat[:, 32:64],
                        rhs=sck[:, mv],
                        start=True,
                        stop=True,
                        tile_position=(0, 64 * pe + 32),
                    )
                stg = gpool.tile([prows, 512 * nbank], f16, tag="s")
                nc.scalar.activation(stg[:], ps[:], Act.Copy)
                nc.gpsimd.dma_start(
                    out=out_d[c, 0:prows, 0 : 512 * nbank], in_=stg[:]
                )

    return nc


def _get_nc():
    if "nc" not in _CACHE:
        nc = _build_nc()
        nc.finalize()  # Bacc: reg-alloc + codegen passes + freeze
        _CACHE["nc"] = nc
    return _CACHE["nc"]


def make_in_maps(input_state, candidate_docs, W1, b1, W2, b2, W3, b3):
    """Shard + lay out host inputs for the 8 cores."""
    blockones = np.zeros((PDIM, PACK), np.float16)
    for g in range(PACK):
        blockones[D * g : D * (g + 1), g] = 1.0
    common = {
        "input_state": np.ascontiguousarray(
            input_state.reshape(D, 1).astype(np.float32)
        ),
        "W1T": np.ascontiguousarray(W1.T.astype(np.float32)),
        "b1r": np.ascontiguousarray(b1.astype(np.float32).reshape(2, 128).T),
        "W2T": np.ascontiguousarray(W2.T.astype(np.float32)),
        "b2r": np.ascontiguousarray(b2.astype(np.float32).reshape(2, 128).T),
        "W3T": np.ascontiguousarray(W3.T.astype(np.float32)),
        "b3t": np.ascontiguousarray(b3.astype(np.float32).reshape(SLATE, D).T),
        "blockones": blockones,
    }
    in_maps = []
    for i in range(NCORES):
        shard = candidate_docs[i * SHARD : (i + 1) * SHARD]
        pad = np.zeros((PAD_SHARD, D), np.float16)
        pad[:SHARD] = shard.astype(np.float16)
        docsT = np.zeros((128, NCOL), np.float16)
        docsT[:PDIM] = pad.reshape(NCOL, PDIM).T
        in_maps.append({"docsT": docsT, **common})
    return in_maps


def decode_scores(out_arr):
    """[NCHUNK, 128, CHUNK//2] fp16 device output -> per-core scores [SHARD, 5]."""
    dots = np.empty((NCOL, PACK, SLATE), np.float32)
    norms = np.empty((NCOL, PACK), np.float32)
    for c in range(NCHUNK):
        cols = CHUNK if c < NFULL else TAIL
        nslice = cols // 512
        nbank = (nslice + 1) // 2
        blk = (
            out_arr[c][:, : 512 * nbank]
            .astype(np.float32)
            .reshape(128, nbank, 512)
        )
        for s in range(nslice):
            pe = s % 2
            bank = s // 2
            n0 = CHUNK * c + 512 * s
            d = blk[64 * pe : 64 * pe + 30, bank].reshape(PACK, SLATE, 512)
            nr = blk[64 * pe + 32 : 64 * pe + 38, bank]
            dots[n0 : n0 + 512] = d.transpose(2, 0, 1)
            norms[n0 : n0 + 512] = nr.T
    scores = norms[:, :, None] - 2.0 * dots
    return scores.reshape(PAD_SHARD, SLATE)[:SHARD]


def run_device(inputs):
    """Run the 8-core kernel; returns (scores [N_DOCS, 5] fp32, results)."""
    from concourse.bass_utils import run_bass_kernel_spmd

    nc = _get_nc()
    in_maps = make_in_maps(**inputs)
    res = run_bass_kernel_spmd(nc, in_maps, core_ids=list(range(NCORES)))
    parts = [decode_scores(np.asarray(res.results[i]["out"])) for i in range(NCORES)]
    return np.concatenate(parts, axis=0), res


def finalize(scores, input_state, candidate_docs, W1, b1, W2, b2, W3, b3):
    """Host pool + exact rerank replicating the reference ops bitwise."""
    import jax
    import jax.numpy as jnp

    pool = []
    for j in range(SLATE):
        pool.append(np.argpartition(scores[:, j], POOL_K)[:POOL_K])
    pool = np.unique(np.concatenate(pool))  # sorted ascending
    cpu = jax.devices("cpu")[0]
    with jax.default_device(cpu):
        x = jnp.asarray(input_state)
        for W, b in ((W1, b1), (W2, b2), (W3, b3)):
            x = jax.nn.leaky_relu(x @ jnp.asarray(W).T + jnp.asarray(b),
                                  negative_slope=SLOPE)
        proto_slate = x.reshape(SLATE, D)
        docs_pool = jnp.asarray(candidate_docs[pool])
        diff = docs_pool[None, :, :] - proto_slate[:, None, :]
        dists = jnp.sqrt(jnp.sum(diff * diff, axis=-1))
        _, idx = jax.lax.top_k(-dists, TOPK)
        idx = np.asarray(idx)
    indices = pool[idx].reshape(-1).astype(np.int32)
    candidates = candidate_docs[indices]
    return candidates, indices


def kernel(**inputs):
    inputs = {k: np.asarray(v) for k, v in inputs.items()}
    scores, _ = run_device(inputs)
    return finalize(scores, **inputs)


# revision 17
# speedup vs baseline: 1.5146x; 1.3783x over previous
"""Distributed KNN-retrieval kernel for 8 Trainium2 NeuronCores.

Reference computation:
  protos = MLP(input_state)                 # [5, 20]
  dists[s, n] = || candidate_docs[n] - protos[s] ||_2
  top-100 smallest per slate -> (candidates [500, 20], indices [500])

Device strategy (SPMD over 8 cores, candidate_docs row-sharded):
  - Host pre-packs each 125k-row shard as fp16 [128, 20992] (rows 0-119 carry
    6 docs x 20 dims per column; rows 120-127 are zero pad -- full-128-partition
    DMA is ~2.2x faster than 120-partition DMA on this part).
  - On-device MLP (fp32 PE + scalar-engine leaky-relu) reproduces the reference
    policy net and emits the proto matrix transposed [20, 5].
  - Stationary A [120, 32]: 6 copies of protoT on the block diagonal.
    Stationary B [120, 32]: block-ones.  A matmul of A against a [120, 512]
    moving slice yields dot products for 6 docs x 5 slates; B against the
    squared docs yields the 6 doc norms.  Dots and norms go to SEPARATE psum
    banks so norm matmuls (no MLP dependency) keep the PE busy while the
    MLP/stationary-A chain resolves.
  - Scores s = ||c||^2 - 2 c.p are assembled on host (fp16 is plenty for
    candidate SELECTION: validated max local rank of a true top-100 member is
    ~20), then the final top-100 is recomputed exactly (same jnp ops as the
    reference) over a pooled candidate set, so the returned indices and rows
    match the reference bitwise.
"""

import sys

sys.path.insert(0, "/opt/trn_rl_repo")

import numpy as np

# ---- problem constants (hardcoded per harness contract) ----
D = 20
SLATE = 5
TOPK = 100
N_DOCS = 1_000_000
HID = 256
SLOPE = 0.01

NCORES = 8
SHARD = N_DOCS // NCORES          # 125000
PACK = 6                          # docs packed per column
PDIM = PACK * D                   # 120 data partitions (+8 pad)
NCOL = 20992                      # columns per core: 6*20992 = 125952 >= 125000
PAD_SHARD = PACK * NCOL           # 125952
CHUNK = 4096                      # moving columns per DMA chunk
NFULL = 5                         # full chunks; tail = 512
TAIL = NCOL - NFULL * CHUNK       # 512
NCHUNK = NFULL + 1
POOL_K = 2048                     # per-slate host pool size for exact rerank

# weights-blob column offsets (f32, [128, WBCOLS])
WB_W2 = 0          # + 256*k + c          (c < 256)
WB_W3 = 512        # + 100*k + 20*j + c   (c < 20)
WB_B1 = 712        # + m
WB_B2 = 714        # + m
WB_W1 = 716        # + c                  (rows 0-19, c < 256)
WB_X = 972         # rows 0-19
WB_B3 = 973        # + j                  (rows 0-19)
WBCOLS = 978

_CACHE = {}


def _build_nc():
    """Build the per-core Bass/Tile graph (identical on all 8 cores)."""
    from concourse import bacc, mybir
    import concourse.tile as tile

    f32 = mybir.dt.float32
    f16 = mybir.dt.float16
    Act = mybir.ActivationFunctionType
    Alu = mybir.AluOpType

    nc = bacc.Bacc("TRN2", target_bir_lowering=False)

    docsT = nc.declare_dram_parameter("docsT", [128, NCOL], f16, isOutput=False)
    wb_in = nc.declare_dram_parameter("wblob", [128, WBCOLS], f32, isOutput=False)
    ones_in = nc.declare_dram_parameter("blockones", [PDIM, PACK], f16, isOutput=False)
    out_d = nc.declare_dram_parameter(
        "out", [NCHUNK, 128, CHUNK // 2], f16, isOutput=True
    )

    with tile.TileContext(nc) as tc:
        with (
            tc.tile_pool(name="const", bufs=1) as const,
            tc.tile_pool(name="mlp", bufs=1) as mlp,
            tc.tile_pool(name="docs", bufs=3) as dpool,
            tc.tile_pool(name="sq", bufs=3) as qpool,
            tc.tile_pool(name="stg", bufs=3) as gpool,
            tc.tile_pool(name="ps", bufs=8, space="PSUM") as pspool,
        ):
            # ---------------- setup ----------------
            wb = const.tile([128, WBCOLS], f32)
            nc.sync.dma_start(out=wb[:], in_=wb_in[:])

            # stationary [120, 64]: cols 0-29 proto block-diagonal (30-31 zero),
            # cols 32-37 block-ones (38-63 zero); both matmul stationaries are
            # 32 wide so full 32-partition quarters of PSUM get written.
            stat = const.tile([PDIM, 64], f16)
            nc.vector.memset(stat[:], 0.0)
            nc.scalar.dma_start(out=stat[:, 32 : 32 + PACK], in_=ones_in[:])

            def leaky(dst, src):
                t = mlp.tile(list(src.shape), f32, tag="lk", name="lk")
                nc.vector.tensor_scalar_mul(t[:], src, SLOPE)
                nc.vector.tensor_tensor(out=dst, in0=src, in1=t[:], op=Alu.max)

            # ---------------- MLP (fp32) ----------------
            # layer 1+2 activations as [128, 2] (col m = units 128m..128m+127)
            ps1 = pspool.tile([128, 2], f32, tag="ps")
            for m in range(2):
                nc.tensor.matmul(
                    out=ps1[:, m : m + 1],
                    lhsT=wb[0:D, WB_W1 + 128 * m : WB_W1 + 128 * (m + 1)],
                    rhs=wb[0:D, WB_X : WB_X + 1],
                    start=True,
                    stop=True,
                )
            z1 = mlp.tile([128, 2], f32, tag="z1")
            nc.vector.tensor_tensor(
                out=z1[:], in0=ps1[:], in1=wb[:, WB_B1 : WB_B1 + 2], op=Alu.add
            )
            h1 = mlp.tile([128, 2], f32, tag="h1")
            leaky(h1[:], z1[:])

            ps2 = pspool.tile([128, 2], f32, tag="ps")
            for m in range(2):
                for k in range(2):
                    nc.tensor.matmul(
                        out=ps2[:, m : m + 1],
                        lhsT=wb[:, WB_W2 + 256 * k + 128 * m : WB_W2 + 256 * k + 128 * (m + 1)],
                        rhs=h1[:, k : k + 1],
                        start=(k == 0),
                        stop=(k == 1),
                    )
            z2 = mlp.tile([128, 2], f32, tag="z2")
            nc.vector.tensor_tensor(
                out=z2[:], in0=ps2[:], in1=wb[:, WB_B2 : WB_B2 + 2], op=Alu.add
            )
            h2 = mlp.tile([128, 2], f32, tag="h2")
            leaky(h2[:], z2[:])

            # layer 3: emit protoT [20, 5] directly (column j = proto_j)
            ps3 = pspool.tile([D, SLATE], f32, tag="ps")
            for j in range(SLATE):
                for k in range(2):
                    nc.tensor.matmul(
                        out=ps3[:, j : j + 1],
                        lhsT=wb[:, WB_W3 + 100 * k + D * j : WB_W3 + 100 * k + D * (j + 1)],
                        rhs=h2[:, k : k + 1],
                        start=(k == 0),
                        stop=(k == 1),
                    )
            z3 = mlp.tile([D, SLATE], f32, tag="z3")
            nc.vector.tensor_tensor(
                out=z3[:], in0=ps3[:], in1=wb[0:D, WB_B3 : WB_B3 + SLATE], op=Alu.add
            )
            pt32 = mlp.tile([D, SLATE], f32, tag="pt32")
            leaky(pt32[:], z3[:])
            pt16 = mlp.tile([D, SLATE], f16, tag="pt16")
            nc.vector.tensor_copy(out=pt16[:], in_=pt32[:])

            # place 6 copies of protoT on the block diagonal (2 engines)
            for g in range(PACK):
                eng = nc.scalar if g % 2 == 0 else nc.gpsimd
                eng.dma_start(
                    out=stat[D * g : D * (g + 1), SLATE * g : SLATE * (g + 1)],
                    in_=pt16[:],
                )

            # ---------------- main loop over doc chunks ----------------
            for c in range(NCHUNK):
                cols = CHUNK if c < NFULL else TAIL
                nslice = cols // 512
                nhalf = (nslice + 3) // 4          # psum tiles per kind
                dck = dpool.tile([128, cols], f16, tag="d")
                nc.sync.dma_start(
                    out=dck[:], in_=docsT[:, CHUNK * c : CHUNK * c + cols]
                )
                sck = qpool.tile([PDIM, cols], f16, tag="q")
                nc.vector.tensor_tensor(
                    out=sck[:], in0=dck[0:PDIM, :], in1=dck[0:PDIM, :], op=Alu.mult
                )

                srows = 128 if nslice >= 4 else 32 * nslice
                stg = gpool.tile([srows, 1024 * nhalf], f16, tag="s")
                psD = [
                    pspool.tile([srows, 512], f32, tag="ps", name=f"psD{c}_{h}")
                    for h in range(nhalf)
                ]
                psN = [
                    pspool.tile([srows, 512], f32, tag="ps", name=f"psN{c}_{h}")
                    for h in range(nhalf)
                ]
                for s in range(nslice):
                    half, q = s // 4, s % 4
                    mv = slice(512 * s, 512 * (s + 1))
                    nc.tensor.matmul(
                        out=psD[half][32 * q : 32 * q + 32, :],
                        lhsT=stat[:, 0:32],
                        rhs=dck[0:PDIM, mv],
                        start=True,
                        stop=True,
                        tile_position=(0, 32 * q),
                    )
                    nc.tensor.matmul(
                        out=psN[half][32 * q : 32 * q + 32, :],
                        lhsT=stat[:, 32:64],
                        rhs=sck[:, mv],
                        start=True,
                        stop=True,
                        tile_position=(0, 32 * q),
                    )
                # staging layout: [dots half0 | dots half1 | norms half0 | norms half1]
                for h in range(nhalf):
                    nc.scalar.activation(
                        stg[:, 512 * h : 512 * (h + 1)], psD[h][:], Act.Copy
                    )
                    nc.scalar.activation(
                        stg[:, 512 * (nhalf + h) : 512 * (nhalf + h + 1)],
                        psN[h][:],
                        Act.Copy,
                    )
                nc.gpsimd.dma_start(
                    out=out_d[c, 0:srows, 0 : 1024 * nhalf], in_=stg[:]
                )

    return nc


def _get_nc():
    if "nc" not in _CACHE:
        nc = _build_nc()
        nc.finalize()  # Bacc: reg-alloc + codegen passes + freeze
        _CACHE["nc"] = nc
    return _CACHE["nc"]


def make_in_maps(input_state, candidate_docs, W1, b1, W2, b2, W3, b3):
    """Shard + lay out host inputs for the 8 cores."""
    blockones = np.zeros((PDIM, PACK), np.float16)
    for g in range(PACK):
        blockones[D * g : D * (g + 1), g] = 1.0
    wb = np.zeros((128, WBCOLS), np.float32)
    W1, W2, W3 = (np.asarray(a, np.float32) for a in (W1, W2, W3))
    b1, b2, b3 = (np.asarray(a, np.float32) for a in (b1, b2, b3))
    for k in range(2):
        wb[:, WB_W2 + 256 * k : WB_W2 + 256 * (k + 1)] = W2.T[128 * k : 128 * (k + 1), :]
        wb[:, WB_W3 + 100 * k : WB_W3 + 100 * (k + 1)] = W3.T[128 * k : 128 * (k + 1), :]
    wb[:, WB_B1] = b1[0:128]
    wb[:, WB_B1 + 1] = b1[128:256]
    wb[:, WB_B2] = b2[0:128]
    wb[:, WB_B2 + 1] = b2[128:256]
    wb[0:D, WB_W1 : WB_W1 + HID] = W1.T
    wb[0:D, WB_X] = np.asarray(input_state, np.float32)
    wb[0:D, WB_B3 : WB_B3 + SLATE] = b3.reshape(SLATE, D).T
    in_maps = []
    for i in range(NCORES):
        shard = candidate_docs[i * SHARD : (i + 1) * SHARD]
        pad = np.zeros((PAD_SHARD, D), np.float16)
        pad[:SHARD] = shard.astype(np.float16)
        docsT = np.zeros((128, NCOL), np.float16)
        docsT[:PDIM] = pad.reshape(NCOL, PDIM).T
        in_maps.append({"docsT": docsT, "wblob": wb, "blockones": blockones})
    return in_maps


def decode_scores(out_arr):
    """[NCHUNK, 128, CHUNK//2] fp16 device output -> per-core scores [SHARD, 5]."""
    dots = np.empty((NCOL, PACK, SLATE), np.float32)
    norms = np.empty((NCOL, PACK), np.float32)
    for c in range(NCHUNK):
        cols = CHUNK if c < NFULL else TAIL
        nslice = cols // 512
        nhalf = (nslice + 3) // 4
        blk = out_arr[c].astype(np.float32)
        for s in range(nslice):
            half, q = s // 4, s % 4
            n0 = CHUNK * c + 512 * s
            dblk = blk[32 * q : 32 * q + 30, 512 * half : 512 * (half + 1)]
            nblk = blk[32 * q : 32 * q + 6, 512 * (nhalf + half) : 512 * (nhalf + half + 1)]
            dots[n0 : n0 + 512] = dblk.reshape(PACK, SLATE, 512).transpose(2, 0, 1)
            norms[n0 : n0 + 512] = nblk.T
    scores = norms[:, :, None] - 2.0 * dots
    return scores.reshape(PAD_SHARD, SLATE)[:SHARD]


def run_device(inputs):
    """Run the 8-core kernel; returns (scores [N_DOCS, 5] fp32, results)."""
    from concourse.bass_utils import run_bass_kernel_spmd

    nc = _get_nc()
    in_maps = make_in_maps(**inputs)
    res = run_bass_kernel_spmd(nc, in_maps, core_ids=list(range(NCORES)))
    parts = [decode_scores(np.asarray(res.results[i]["out"])) for i in range(NCORES)]
    return np.concatenate(parts, axis=0), res


def finalize(scores, input_state, candidate_docs, W1, b1, W2, b2, W3, b3):
    """Host pool + exact rerank replicating the reference ops bitwise."""
    import jax
    import jax.numpy as jnp

    pool = []
    for j in range(SLATE):
        pool.append(np.argpartition(scores[:, j], POOL_K)[:POOL_K])
    pool = np.unique(np.concatenate(pool))  # sorted ascending
    cpu = jax.devices("cpu")[0]
    with jax.default_device(cpu):
        x = jnp.asarray(input_state)
        for W, b in ((W1, b1), (W2, b2), (W3, b3)):
            x = jax.nn.leaky_relu(x @ jnp.asarray(W).T + jnp.asarray(b),
                                  negative_slope=SLOPE)
        proto_slate = x.reshape(SLATE, D)
        docs_pool = jnp.asarray(candidate_docs[pool])
        diff = docs_pool[None, :, :] - proto_slate[:, None, :]
        dists = jnp.sqrt(jnp.sum(diff * diff, axis=-1))
        _, idx = jax.lax.top_k(-dists, TOPK)
        idx = np.asarray(idx)
    indices = pool[idx].reshape(-1).astype(np.int32)
    candidates = candidate_docs[indices]
    return candidates, indices


def kernel(**inputs):
    inputs = {k: np.asarray(v) for k, v in inputs.items()}
    scores, _ = run_device(inputs)
    return finalize(scores, **inputs)
